# revision 22
# baseline (speedup 1.0000x reference)
"""ArcFace loss on 8 TRN2 NeuronCores (Bass/Tile).

Strategy (model-parallel classification head, device kernel = pure
matmul+exp stream):
  - Host: l2-normalize embeddings and weights (fp32), quantize to fp8
    (e_hat*32, w_hat*128), shard classes across 8 cores (12500/core,
    zero-padded to 12544), and compute the per-row target-class
    corrections (ArcFace margin) in float64 from the fp32 inputs.
  - Device (per core): cosine slice = e_hat @ w_hat_local^T on the
    TensorEngine (fp8 DoubleRow), then one-pass ACT exp with accum_out
    over 2048-column PSUM groups: S_loc[b] = sum_c exp(64*cos). scale
    64/(32*128) = 2^-6 is an exact immediate. No max-stabilization
    needed (|64*cos| <= ~64 fits fp32/bf16 comfortably).
  - Host: S[b] = sum over cores of S_loc (minus the 352 zero-pad
    columns that each contribute exp(0)=1), swap in the margin target
    term, loss = mean(log(S') - l_m). No on-device collectives: the
    cross-core reduction is 8 x 4KB, cheaper on host than a ~29us
    mesh AllReduce.

Measured on 8-core trn2 (harness contract: kernel(**inputs) takes the
FULL inputs, returns the full scalar output): 127.0us, rel err 1.2e-4.
Alternatives measured slower on hardware and reverted:
  - DVE/Schraudolph exp offload (bf16 bit-trick): the reduce-variant
    DVE ops run 1 elem/cycle (no 2x/4x modes), so a DVE-drained tile
    costs 4.6us vs ACT's 2.25us, and mixed-drain pipelines pay an
    extra PSUM-coupling stall tax on the PE stream (129-146us).
  - GPSIMD as a third drain engine: cannot read PSUM and cannot run
    reduce/accum ops.
  - Wider matmuls (out free > 512): rejected by the ISA (PSUM bank).
  - On-device AllReduce: ~29us latency + ~38us barrier for 2KB.
"""

import math

import numpy as np
import ml_dtypes

import concourse.bass as bass
import concourse.mybir as mybir
import concourse.tile as tile
from concourse import bacc

AF = mybir.ActivationFunctionType
ALU = mybir.AluOpType
AX = mybir.AxisListType
F32 = mybir.dt.float32
BF16 = mybir.dt.bfloat16
FP8 = mybir.dt.float8e4

MARGIN = 0.5
SCALE = 64.0
EPS = 1e-7

S_E = 32.0
S_W = 128.0


def make_cfg(n_cores=8, b=1024, d=512, c_total=100000):
    c_local = c_total // n_cores
    c_pad = ((c_local + 127) // 128) * 128
    grp_w = []
    rem = c_pad
    while rem > 0:
        g = min(2048, rem)
        grp_w.append(g)
        rem -= g
    return dict(
        n_cores=n_cores,
        b=b,
        d=d,
        c_total=c_total,
        c_local=c_local,
        c_pad=c_pad,
        grp_w=grp_w,
    )


def _drains(cfg):
    """(gi, bo) -> 'A' (ACT exp+accum) | 'E' (ACT exp, DVE sums scr)."""
    out = {}
    t = 0
    for gi, gw in enumerate(cfg["grp_w"]):
        for bo in range(cfg["b"] // 128):
            if gw < 2048:
                out[(gi, bo)] = "A"
            else:
                out[(gi, bo)] = "A" if t % 6 == 5 else "E"
                t += 1
    return out


def build_nc(cfg):
    n_cores = cfg["n_cores"]
    b, d = cfg["b"], cfg["d"]
    c_pad = cfg["c_pad"]
    grp_w = cfg["grp_w"]
    NG = len(grp_w)
    grp_off = [0]
    for gw in grp_w:
        grp_off.append(grp_off[-1] + gw)
    KO = d // 128
    BO = b // 128
    P = 128
    drains = _drains(cfg)

    nc = bacc.Bacc(
        "TRN2",
        target_bir_lowering=False,
        debug=False,
        enable_asserts=True,
        num_devices=n_cores,
    )

    wt_d = nc.dram_tensor("wt", [P, KO * c_pad], FP8, kind="ExternalInput")
    et_d = nc.dram_tensor("et", [P, KO * b], FP8, kind="ExternalInput")
    out_d = nc.dram_tensor("out", [P, BO * NG], F32, kind="ExternalOutput")

    with tile.TileContext(nc) as tc:
        with (
            tc.tile_pool(name="big", bufs=1) as pb,
            tc.tile_pool(name="wpool", bufs=NG) as pw,
            tc.tile_pool(name="scr", bufs=16) as pscr,
            tc.tile_pool(name="jnkp", bufs=2) as pjnk,
            tc.tile_pool(name="small", bufs=1) as ps,
            tc.tile_pool(name="ps_all", bufs=2, space="PSUM") as pps,
        ):
            # ---- load replicated embeddings on the scalar queue (ACT
            # is idle until the first exp); k-split so kp=0 matmuls can
            # start after the first 0.25MB lands ----
            et_sb = pb.tile([P, KO, b], FP8, tag="et")
            et_src = et_d.ap().rearrange("p (k b) -> p k b", k=KO)
            nc.scalar.dma_start(et_sb[:, 0:2, :], et_src[:, 0:2, :])
            nc.scalar.dma_start(et_sb[:, 2:4, :], et_src[:, 2:4, :])
            # ---- prefetch all weight groups across two DMA queues ----
            w_tiles = []
            for gi in range(NG):
                gw = grp_w[gi]
                c0 = grp_off[gi]
                Wg = pw.tile([P, KO, 2048], FP8, tag="Wg")
                w_tiles.append(Wg)
                q = nc.sync if gi % 2 == 0 else nc.gpsimd
                q.dma_start(
                    Wg[:, :, :gw],
                    wt_d.ap()[:, KO * c0 : KO * (c0 + gw)].rearrange(
                        "p (k n) -> p k n", k=KO
                    ),
                )

            sums = ps.tile([P, BO, NG], F32, tag="sums")
            # process the small (256-wide) group mid-stream so its 8
            # short serial exps hide under the big-tile pipeline instead
            # of dangling as pure tail after the final matmul
            order = [0, 1, 2, 3, 6, 4, 5]
            assert sorted(order) == list(range(NG))
            for gi in order:
                gw = grp_w[gi]
                Wg = w_tiles[gi]
                for bo in range(BO):
                    bs = slice(bo * P, (bo + 1) * P)
                    psm = pps.tile([P, 2048], F32, tag="ps")
                    for kp in range(KO // 2):
                        ks = slice(2 * kp, 2 * kp + 2)
                        for o in range(0, gw, 512):
                            nw = min(512, gw - o)
                            nc.tensor.matmul(
                                psm[:, o : o + nw],
                                et_sb[:, ks, bs],
                                Wg[:, ks, o : o + nw],
                                start=(kp == 0),
                                stop=(kp == KO // 2 - 1),
                                perf_mode=mybir.MatmulPerfMode.DoubleRow,
                            )
                    slot = sums[:, bo, gi : gi + 1]
                    scr = pscr.tile([P, 2048], BF16, tag="escr")
                    if drains[(gi, bo)] == "A":
                        nc.scalar.activation(
                            scr[:, :gw],
                            psm[:, :gw],
                            AF.Exp,
                            scale=SCALE / (S_E * S_W),
                            accum_out=slot,
                        )
                    else:
                        # ACT exp only (no accumulator read); the DVE sums
                        # the bf16 result off the critical path. The deep
                        # scr ring (16 bufs) keeps ACT from ever waiting
                        # on the slower DVE sum stream.
                        nc.scalar.activation(
                            scr[:, :gw],
                            psm[:, :gw],
                            AF.Exp,
                            scale=SCALE / (S_E * S_W),
                        )
                        jnk = pjnk.tile([P, 2048], BF16, tag="jnk")
                        nc.vector.tensor_scalar(
                            jnk[:, :gw], scr[:, :gw],
                            1.0, 0.0, ALU.mult, ALU.add,
                            accum_out=slot,
                        )

            nc.sync.dma_start(
                out_d.ap(), sums[:].rearrange("p b g -> p (b g)")
            )

    nc.compile()
    return nc


def prep_inputs(cfg, embeddings, weight):
    """Normalize + quantize + shard the full inputs into per-core in_maps."""
    n_cores = cfg["n_cores"]
    b, d = cfg["b"], cfg["d"]
    c_local, c_pad = cfg["c_local"], cfg["c_pad"]
    KO = d // 128
    P = 128

    e = np.asarray(embeddings, np.float32)
    w = np.asarray(weight, np.float32)
    e_hat = e / np.maximum(
        np.linalg.norm(e, axis=-1, keepdims=True), 1e-12
    )
    w_hat = w / np.maximum(
        np.linalg.norm(w, axis=-1, keepdims=True), 1e-12
    )

    # replicated transposed embeddings: [d, b] -> [P, KO, b] part-major
    et = (e_hat.T * S_E).astype(ml_dtypes.float8_e4m3)
    et_host = np.ascontiguousarray(
        et.reshape(KO, P, b).transpose(1, 0, 2).reshape(P, KO * b)
    )

    in_maps = []
    for i in range(n_cores):
        ws = w_hat[i * c_local : (i + 1) * c_local]
        if c_pad > c_local:
            ws = np.concatenate(
                [ws, np.zeros((c_pad - c_local, d), np.float32)], axis=0
            )
        wt = (ws * S_W).astype(ml_dtypes.float8_e4m3).T  # [d, c_pad]
        wt4 = np.ascontiguousarray(wt).reshape(KO, P, c_pad)
        blocks = []
        c0 = 0
        for gw in cfg["grp_w"]:
            blk = wt4[:, :, c0 : c0 + gw]  # [KO, P, gw]
            blocks.append(blk.transpose(1, 0, 2).reshape(P, KO * gw))
            c0 += gw
        wt_host = np.ascontiguousarray(np.concatenate(blocks, axis=1))
        in_maps.append({"wt": wt_host, "et": et_host})
    return in_maps, e_hat, w_hat


_CACHED = {}


def _get_nc(cfg_key, cfg):
    if cfg_key not in _CACHED:
        _CACHED[cfg_key] = build_nc(cfg)
    return _CACHED[cfg_key]


def run(inputs, mm_dtype="fp8", trace=False, **kw):
    from concourse.bass_utils import run_bass_kernel_spmd

    cfg = make_cfg()
    nc = _get_nc((mm_dtype,), cfg)
    in_maps, e_hat, w_hat = prep_inputs(
        cfg, inputs["embeddings"], inputs["weight"]
    )
    res = run_bass_kernel_spmd(
        nc, in_maps, core_ids=list(range(cfg["n_cores"])), trace=trace, **kw
    )

    b = cfg["b"]
    BO = b // 128
    NG = len(cfg["grp_w"])
    # slot[p, bo, gi] holds rows b = bo*128 + p
    S = np.zeros(b, np.float64)
    for i in range(cfg["n_cores"]):
        slots = res.results[i]["out"].astype(np.float64).reshape(128, BO, NG)
        S += slots.sum(axis=2).T.reshape(-1)
    # each core's (c_pad - c_local) zero-pad columns contribute exp(0) = 1
    S -= float(cfg["n_cores"] * (cfg["c_pad"] - cfg["c_local"]))

    # target-class margin correction (float64, exact w.r.t. fp32 inputs)
    labels = np.asarray(inputs["labels"]).astype(np.int64)
    cos_t = np.einsum(
        "bd,bd->b",
        e_hat.astype(np.float64),
        w_hat[labels].astype(np.float64),
    )
    cos_c = np.clip(cos_t, -1.0 + EPS, 1.0 - EPS)
    theta = np.arccos(cos_c)
    l_t = SCALE * cos_t
    l_m = SCALE * np.cos(theta + MARGIN)
    S2 = S - np.exp(l_t) + np.exp(l_m)
    loss = np.mean(np.log(S2) - l_m)
    return np.float32(loss), res


def kernel(**inputs):
    loss, _ = run(inputs, trace=False)
    return np.asarray(loss, dtype=np.float32).reshape(())


# revision 23
# speedup vs baseline: 1.1096x; 1.1096x over previous
"""ArcFace loss on 8 TRN2 NeuronCores (Bass/Tile).

Strategy (model-parallel classification head, device kernel = pure
matmul+exp stream):
  - Host: l2-normalize embeddings and weights (fp32), quantize to fp8
    (e_hat*32, w_hat*128), shard classes across 8 cores (12500/core,
    zero-padded to 12544), and compute the per-row target-class
    corrections (ArcFace margin) in float64 from the fp32 inputs.
  - Device (per core): cosine slice = e_hat @ w_hat_local^T on the
    TensorEngine (fp8 DoubleRow), then one-pass ACT exp with accum_out
    over 2048-column PSUM groups: S_loc[b] = sum_c exp(64*cos). scale
    64/(32*128) = 2^-6 is an exact immediate. No max-stabilization
    needed (|64*cos| <= ~64 fits fp32/bf16 comfortably).
  - Host: S[b] = sum over cores of S_loc (minus the 352 zero-pad
    columns that each contribute exp(0)=1), swap in the margin target
    term, loss = mean(log(S') - l_m). No on-device collectives: the
    cross-core reduction is 8 x 4KB, cheaper on host than a ~29us
    mesh AllReduce.

Measured on 8-core trn2 (harness contract: kernel(**inputs) takes the
FULL inputs, returns the full scalar output): 122.9us, rel err 1.2e-4.
Most full tiles drain as 'E': ACT exp (no accumulator read) into a
deep 16-buffer bf16 scr ring, with the DVE summing each scr tile off
the critical path; every 6th tile plus the small group stays fully on
ACT ('A', exp+accum_out) to balance the two engines (~98us ACT /
~92us DVE busy against a ~94us TensorE stream).
Alternatives measured slower on hardware and reverted:
  - DVE/Schraudolph exp offload (bf16 bit-trick): the reduce-variant
    DVE ops run 1 elem/cycle (no 2x/4x modes), so a DVE-drained tile
    costs 4.6us vs ACT's 2.25us, and mixed-drain pipelines pay an
    extra PSUM-coupling stall tax on the PE stream (129-146us).
  - GPSIMD as a third drain engine: cannot read PSUM and cannot run
    reduce/accum ops.
  - Wider matmuls (out free > 512): rejected by the ISA (PSUM bank).
  - On-device AllReduce: ~29us latency + ~38us barrier for 2KB.
"""

import math

import numpy as np
import ml_dtypes

import concourse.bass as bass
import concourse.mybir as mybir
import concourse.tile as tile
from concourse import bacc

AF = mybir.ActivationFunctionType
ALU = mybir.AluOpType
AX = mybir.AxisListType
F32 = mybir.dt.float32
BF16 = mybir.dt.bfloat16
FP8 = mybir.dt.float8e4

MARGIN = 0.5
SCALE = 64.0
EPS = 1e-7

S_E = 32.0
S_W = 128.0


def make_cfg(n_cores=8, b=1024, d=512, c_total=100000):
    c_local = c_total // n_cores
    c_pad = ((c_local + 127) // 128) * 128
    grp_w = []
    rem = c_pad
    while rem > 0:
        g = min(2048, rem)
        grp_w.append(g)
        rem -= g
    return dict(
        n_cores=n_cores,
        b=b,
        d=d,
        c_total=c_total,
        c_local=c_local,
        c_pad=c_pad,
        grp_w=grp_w,
    )


def _drains(cfg):
    """(gi, bo) -> 'A' (ACT exp+accum) | 'E' (ACT exp, DVE sums scr)."""
    out = {}
    t = 0
    for gi, gw in enumerate(cfg["grp_w"]):
        for bo in range(cfg["b"] // 128):
            if gw < 2048:
                out[(gi, bo)] = "A"
            else:
                out[(gi, bo)] = "A" if t % 6 == 5 else "E"
                t += 1
    return out


def build_nc(cfg):
    n_cores = cfg["n_cores"]
    b, d = cfg["b"], cfg["d"]
    c_pad = cfg["c_pad"]
    grp_w = cfg["grp_w"]
    NG = len(grp_w)
    grp_off = [0]
    for gw in grp_w:
        grp_off.append(grp_off[-1] + gw)
    KO = d // 128
    BO = b // 128
    P = 128
    drains = _drains(cfg)

    nc = bacc.Bacc(
        "TRN2",
        target_bir_lowering=False,
        debug=False,
        enable_asserts=True,
        num_devices=n_cores,
    )

    wt_d = nc.dram_tensor("wt", [P, KO * c_pad], FP8, kind="ExternalInput")
    et_d = nc.dram_tensor("et", [P, KO * b], FP8, kind="ExternalInput")
    out_d = nc.dram_tensor("out", [P, BO * NG], F32, kind="ExternalOutput")

    with tile.TileContext(nc) as tc:
        with (
            tc.tile_pool(name="big", bufs=1) as pb,
            tc.tile_pool(name="wpool", bufs=NG) as pw,
            tc.tile_pool(name="scr", bufs=16) as pscr,
            tc.tile_pool(name="jnkp", bufs=2) as pjnk,
            tc.tile_pool(name="small", bufs=1) as ps,
            tc.tile_pool(name="ps_all", bufs=2, space="PSUM") as pps,
        ):
            # ---- load replicated embeddings ----
            et_sb = pb.tile([P, KO, b], FP8, tag="et")
            nc.sync.dma_start(
                et_sb[:], et_d.ap().rearrange("p (k b) -> p k b", k=KO)
            )
            # ---- prefetch all weight groups (DMA runs ahead of compute) ----
            w_tiles = []
            for gi in range(NG):
                gw = grp_w[gi]
                c0 = grp_off[gi]
                Wg = pw.tile([P, KO, 2048], FP8, tag="Wg")
                w_tiles.append(Wg)
                nc.sync.dma_start(
                    Wg[:, :, :gw],
                    wt_d.ap()[:, KO * c0 : KO * (c0 + gw)].rearrange(
                        "p (k n) -> p k n", k=KO
                    ),
                )

            sums = ps.tile([P, BO, NG], F32, tag="sums")
            for gi in range(NG):
                gw = grp_w[gi]
                Wg = w_tiles[gi]
                for bo in range(BO):
                    bs = slice(bo * P, (bo + 1) * P)
                    psm = pps.tile([P, 2048], F32, tag="ps")
                    for kp in range(KO // 2):
                        ks = slice(2 * kp, 2 * kp + 2)
                        for o in range(0, gw, 512):
                            nw = min(512, gw - o)
                            nc.tensor.matmul(
                                psm[:, o : o + nw],
                                et_sb[:, ks, bs],
                                Wg[:, ks, o : o + nw],
                                start=(kp == 0),
                                stop=(kp == KO // 2 - 1),
                                perf_mode=mybir.MatmulPerfMode.DoubleRow,
                            )
                    slot = sums[:, bo, gi : gi + 1]
                    scr = pscr.tile([P, 2048], BF16, tag="escr")
                    if drains[(gi, bo)] == "A":
                        nc.scalar.activation(
                            scr[:, :gw],
                            psm[:, :gw],
                            AF.Exp,
                            scale=SCALE / (S_E * S_W),
                            accum_out=slot,
                        )
                    else:
                        # ACT exp only (no accumulator read); the DVE sums
                        # the bf16 result off the critical path. The deep
                        # scr ring (16 bufs) keeps ACT from ever waiting
                        # on the slower DVE sum stream.
                        nc.scalar.activation(
                            scr[:, :gw],
                            psm[:, :gw],
                            AF.Exp,
                            scale=SCALE / (S_E * S_W),
                        )
                        jnk = pjnk.tile([P, 2048], BF16, tag="jnk")
                        nc.vector.tensor_scalar(
                            jnk[:, :gw], scr[:, :gw],
                            1.0, 0.0, ALU.mult, ALU.add,
                            accum_out=slot,
                        )

            nc.sync.dma_start(
                out_d.ap(), sums[:].rearrange("p b g -> p (b g)")
            )

    nc.compile()
    return nc


def prep_inputs(cfg, embeddings, weight):
    """Normalize + quantize + shard the full inputs into per-core in_maps."""
    n_cores = cfg["n_cores"]
    b, d = cfg["b"], cfg["d"]
    c_local, c_pad = cfg["c_local"], cfg["c_pad"]
    KO = d // 128
    P = 128

    e = np.asarray(embeddings, np.float32)
    w = np.asarray(weight, np.float32)
    e_hat = e / np.maximum(
        np.linalg.norm(e, axis=-1, keepdims=True), 1e-12
    )
    w_hat = w / np.maximum(
        np.linalg.norm(w, axis=-1, keepdims=True), 1e-12
    )

    # replicated transposed embeddings: [d, b] -> [P, KO, b] part-major
    et = (e_hat.T * S_E).astype(ml_dtypes.float8_e4m3)
    et_host = np.ascontiguousarray(
        et.reshape(KO, P, b).transpose(1, 0, 2).reshape(P, KO * b)
    )

    in_maps = []
    for i in range(n_cores):
        ws = w_hat[i * c_local : (i + 1) * c_local]
        if c_pad > c_local:
            ws = np.concatenate(
                [ws, np.zeros((c_pad - c_local, d), np.float32)], axis=0
            )
        wt = (ws * S_W).astype(ml_dtypes.float8_e4m3).T  # [d, c_pad]
        wt4 = np.ascontiguousarray(wt).reshape(KO, P, c_pad)
        blocks = []
        c0 = 0
        for gw in cfg["grp_w"]:
            blk = wt4[:, :, c0 : c0 + gw]  # [KO, P, gw]
            blocks.append(blk.transpose(1, 0, 2).reshape(P, KO * gw))
            c0 += gw
        wt_host = np.ascontiguousarray(np.concatenate(blocks, axis=1))
        in_maps.append({"wt": wt_host, "et": et_host})
    return in_maps, e_hat, w_hat


_CACHED = {}


def _get_nc(cfg_key, cfg):
    if cfg_key not in _CACHED:
        _CACHED[cfg_key] = build_nc(cfg)
    return _CACHED[cfg_key]


def run(inputs, mm_dtype="fp8", trace=False, **kw):
    from concourse.bass_utils import run_bass_kernel_spmd

    cfg = make_cfg()
    nc = _get_nc((mm_dtype,), cfg)
    in_maps, e_hat, w_hat = prep_inputs(
        cfg, inputs["embeddings"], inputs["weight"]
    )
    res = run_bass_kernel_spmd(
        nc, in_maps, core_ids=list(range(cfg["n_cores"])), trace=trace, **kw
    )

    b = cfg["b"]
    BO = b // 128
    NG = len(cfg["grp_w"])
    # slot[p, bo, gi] holds rows b = bo*128 + p
    S = np.zeros(b, np.float64)
    for i in range(cfg["n_cores"]):
        slots = res.results[i]["out"].astype(np.float64).reshape(128, BO, NG)
        S += slots.sum(axis=2).T.reshape(-1)
    # each core's (c_pad - c_local) zero-pad columns contribute exp(0) = 1
    S -= float(cfg["n_cores"] * (cfg["c_pad"] - cfg["c_local"]))

    # target-class margin correction (float64, exact w.r.t. fp32 inputs)
    labels = np.asarray(inputs["labels"]).astype(np.int64)
    cos_t = np.einsum(
        "bd,bd->b",
        e_hat.astype(np.float64),
        w_hat[labels].astype(np.float64),
    )
    cos_c = np.clip(cos_t, -1.0 + EPS, 1.0 - EPS)
    theta = np.arccos(cos_c)
    l_t = SCALE * cos_t
    l_m = SCALE * np.cos(theta + MARGIN)
    S2 = S - np.exp(l_t) + np.exp(l_m)
    loss = np.mean(np.log(S2) - l_m)
    return np.float32(loss), res


def kernel(**inputs):
    loss, _ = run(inputs, trace=False)
    return np.asarray(loss, dtype=np.float32).reshape(())
